# revision 6
# baseline (speedup 1.0000x reference)
"""GuidedMultiHeadAttention Trainium2 kernel.

Strategy: data-parallel over batch B=8, one batch element per NeuronCore.
Per core: QKV projections, per-head biased-softmax attention (mask bias fused
into the score matmul via an augmented contraction row), attention applied to
V, output projection. Returns (out, attn) like the reference.

All heavy matmuls run in float32r (replicated fp32, ~1 col/cycle for free
dims >= 256). Exp row-sums come free via the scalar engine's accum_out.
"""

import numpy as np

import concourse.bacc as bacc
import concourse.mybir as mybir
import concourse.tile as tile
import concourse.bass as bass
from concourse.bass_utils import run_bass_kernel_spmd
from concourse.masks import make_identity

F32 = mybir.dt.float32
F32R = mybir.dt.float32r
Identity = mybir.ActivationFunctionType.Identity
Exp = mybir.ActivationFunctionType.Exp

B, N, D, H, DH = 8, 785, 768, 12, 64
NP = 788                 # key/position axis padded to a multiple of 4 (fp32r needs 4-aligned free dims)
DT = D // 128            # 6 din/dout tiles
NQ = (N + 127) // 128    # 7 query/key position tiles (last has 17 rows)
KC = [(0, 512), (512, NP - 512)]     # key / position chunks (512, 276)
OC = [(0, 512), (512, D - 512)]      # dout chunks (512, 256)
ALPHA, EPS = 10.0, 1e-6


def _ptile(i):
    return 128 if i < NQ - 1 else N - 128 * (NQ - 1)


def build_kernel():
    nc = bacc.Bacc(None)

    xT = nc.dram_tensor("xT", [D, NP], F32R, kind="ExternalInput")
    wq = nc.dram_tensor("wq", [D, D], F32R, kind="ExternalInput")   # (Wq/8).T (din,dout)
    wk = nc.dram_tensor("wk", [D, D], F32R, kind="ExternalInput")
    wv = nc.dram_tensor("wv", [D, D], F32R, kind="ExternalInput")
    wo = nc.dram_tensor("wo", [D, D], F32R, kind="ExternalInput")
    bq = nc.dram_tensor("bq", [128, DT], F32, kind="ExternalInput")  # (bq/8) pair layout
    bk = nc.dram_tensor("bk", [128, DT], F32, kind="ExternalInput")
    bo = nc.dram_tensor("bo", [1, D], F32, kind="ExternalInput")     # bo + Wo@bv
    bias = nc.dram_tensor("bias", [2, NP], F32R, kind="ExternalInput")  # row0: ALPHA*log(mask+eps) (pad -1e30); row1: ones

    out_d = nc.dram_tensor("out", [N, D], F32, kind="ExternalOutput")
    attn_d = nc.dram_tensor("attn", [H, N, N], F32, kind="ExternalOutput")

    with tile.TileContext(nc) as tc:
        with (
            tc.tile_pool(name="qk", bufs=1) as qkp,       # Q.T/K.T aug tiles, whole kernel
            tc.tile_pool(name="vsb", bufs=1) as vp,       # V tiles, whole kernel
            tc.tile_pool(name="singles", bufs=1) as sg,   # constants
        ):
            qa = [qkp.tile([65, NP], F32R, tag=f"qa{h}", name=f"qa{h}") for h in range(H)]
            ka = [qkp.tile([65, NP], F32R, tag=f"ka{h}", name=f"ka{h}") for h in range(H)]
            v_sb = [vp.tile([128, D], F32R, tag=f"v{pt}", name=f"v{pt}") for pt in range(NQ)]

            bq_sb = sg.tile([128, DT], F32)
            bk_sb = sg.tile([128, DT], F32)
            nc.sync.dma_start(out=bq_sb, in_=bq[:, :])
            nc.sync.dma_start(out=bk_sb, in_=bk[:, :])
            ident = sg.tile([128, 128], F32)
            make_identity(nc, ident)
            bo_sb = sg.tile([128, D], F32)
            nc.sync.dma_start(out=bo_sb, in_=bo[0:1, :].to_broadcast([128, D]))

            for h in range(H):
                nc.sync.dma_start(out=qa[h][64:65, :], in_=bias[1:2, :])
                nc.sync.dma_start(out=ka[h][64:65, :], in_=bias[0:1, :])

            # ---------------- Phase 1: projections ----------------
            with (
                tc.tile_pool(name="xw", bufs=1) as xw,
                tc.tile_pool(name="pq", bufs=2, space="PSUM") as pq,
                tc.tile_pool(name="pk", bufs=2, space="PSUM") as pk,
                tc.tile_pool(name="pv", bufs=2, space="PSUM") as pv,
            ):
                xt = [xw.tile([128, NP], F32R, tag=f"xt{kt}", name=f"xt{kt}") for kt in range(DT)]
                wq_t = [xw.tile([128, D], F32R, tag=f"wq{kt}", name=f"wqt{kt}") for kt in range(DT)]
                wk_t = [xw.tile([128, D], F32R, tag=f"wk{kt}", name=f"wkt{kt}") for kt in range(DT)]
                wv_t = [xw.tile([128, D], F32R, tag=f"wv{kt}", name=f"wvt{kt}") for kt in range(DT)]
                for kt in range(DT):
                    sl = slice(kt * 128, (kt + 1) * 128)
                    nc.sync.dma_start(out=xt[kt], in_=xT[sl, :])
                    nc.sync.dma_start(out=wq_t[kt], in_=wq[sl, :])
                    nc.sync.dma_start(out=wk_t[kt], in_=wk[sl, :])
                    nc.sync.dma_start(out=wv_t[kt], in_=wv[sl, :])

                # Q.T and K.T, two heads at a time (M=128 = head pair)
                for t in range(DT):
                    hs = slice(t * 128, (t + 1) * 128)
                    for (c0, cw) in KC:
                        psq = pq.tile([128, 512], F32, tag="psq")
                        psk = pk.tile([128, 512], F32, tag="psk")
                        for kt in range(DT):
                            nc.tensor.matmul(
                                psq[:, :cw], wq_t[kt][:, hs], xt[kt][:, c0:c0 + cw],
                                start=(kt == 0), stop=(kt == DT - 1))
                        for kt in range(DT):
                            nc.tensor.matmul(
                                psk[:, :cw], wk_t[kt][:, hs], xt[kt][:, c0:c0 + cw],
                                start=(kt == 0), stop=(kt == DT - 1))
                        for half in range(2):
                            hsl = slice(64 * half, 64 * half + 64)
                            nc.scalar.activation(
                                qa[2 * t + half][0:64, c0:c0 + cw], psq[hsl, :cw],
                                Identity, bias=bq_sb[hsl, t:t + 1])
                            nc.scalar.activation(
                                ka[2 * t + half][0:64, c0:c0 + cw], psk[hsl, :cw],
                                Identity, bias=bk_sb[hsl, t:t + 1])

                # V in natural (position, dout) layout: x.T tiles as stationary
                for pt in range(NQ):
                    M = _ptile(pt)
                    p0 = pt * 128
                    for (c0, cw) in OC:
                        psv = pv.tile([128, 512], F32, tag="psv")
                        for kt in range(DT):
                            nc.tensor.matmul(
                                psv[:M, :cw], xt[kt][:, p0:p0 + M], wv_t[kt][:, c0:c0 + cw],
                                start=(kt == 0), stop=(kt == DT - 1))
                        nc.vector.tensor_copy(v_sb[pt][:M, c0:c0 + cw], psv[:M, :cw])

            # ---------------- Phase 2: attention ----------------
            with (
                tc.tile_pool(name="wop", bufs=1) as wop,
                tc.tile_pool(name="utp", bufs=1) as utp,
                tc.tile_pool(name="attnT", bufs=2) as atp,
                tc.tile_pool(name="expp", bufs=3) as expp,
                tc.tile_pool(name="attn", bufs=3) as attp,
                tc.tile_pool(name="small", bufs=8) as smallp,
                tc.tile_pool(name="outp", bufs=2) as outp,
                tc.tile_pool(name="pss", bufs=3, space="PSUM") as pss,
                tc.tile_pool(name="ptr", bufs=2, space="PSUM") as ptr,
                tc.tile_pool(name="psu", bufs=2, space="PSUM") as psup,
            ):
                wo_t = [wop.tile([128, D], F32R, tag=f"wo{kt}", name=f"wot{kt}") for kt in range(DT)]
                for kt in range(DT):
                    nc.sync.dma_start(out=wo_t[kt], in_=wo[kt * 128:(kt + 1) * 128, :])
                ut_sb = [utp.tile([128, NP], F32R, tag=f"ut{t}", name=f"ut{t}") for t in range(DT)]

                for h in range(H):
                    atile = [atp.tile([128, NP], F32R, tag=f"at{kt}", name=f"at{kt}") for kt in range(NQ)]
                    for qt in range(NQ):
                        M = _ptile(qt)
                        q0 = qt * 128
                        expm = expp.tile([128, NP], F32, tag="expm")
                        sums = smallp.tile([128, 4], F32, tag="sums")
                        for i, (c0, cw) in enumerate(KC):
                            ps = pss.tile([128, 512], F32, tag="ps")
                            nc.tensor.matmul(
                                ps[:M, :cw], qa[h][:, q0:q0 + M], ka[h][:, c0:c0 + cw],
                                start=True, stop=True)
                            nc.scalar.activation(
                                expm[:M, c0:c0 + cw], ps[:M, :cw], Exp,
                                accum_out=sums[:M, i:i + 1])
                        rec = smallp.tile([128, 1], F32, tag="rec")
                        nc.vector.tensor_add(rec[:M], sums[:M, 0:1], sums[:M, 1:2])
                        nc.vector.reciprocal(rec[:M], rec[:M])
                        attn_sb = attp.tile([128, NP], F32, tag="attn")
                        nc.vector.tensor_scalar_mul(attn_sb[:M, :], expm[:M, :], rec[:M, 0:1])
                        nc.sync.dma_start(out=attn_d[h, q0:q0 + M, :], in_=attn_sb[:M, 0:N])
                        for kt in range(NQ):
                            KM = _ptile(kt)
                            k0 = kt * 128
                            tps = ptr.tile([128, 128], F32, tag="tps")
                            nc.tensor.transpose(
                                tps[:KM, :M], attn_sb[:M, k0:k0 + KM], ident[:M, :M])
                            nc.vector.tensor_copy(atile[kt][:KM, q0:q0 + M], tps[:KM, :M])

                    # U.T[dh, q] accumulated over key tiles; write into ut_sb rows
                    half = 64 * (h % 2)
                    for (c0, cw) in KC:
                        psu = psup.tile([64, 512], F32, tag="psu")
                        for kt in range(NQ):
                            KM = _ptile(kt)
                            nc.tensor.matmul(
                                psu[:, :cw], v_sb[kt][:KM, h * 64:h * 64 + 64],
                                atile[kt][:KM, c0:c0 + cw],
                                start=(kt == 0), stop=(kt == NQ - 1))
                        nc.vector.tensor_copy(
                            ut_sb[h // 2][half:half + 64, c0:c0 + cw], psu[:, :cw])

                # ---------------- output projection ----------------
                for qt in range(NQ):
                    M = _ptile(qt)
                    q0 = qt * 128
                    ob = outp.tile([128, D], F32, tag="ob")
                    for (c0, cw) in OC:
                        psf = pss.tile([128, 512], F32, tag="ps")
                        for t in range(DT):
                            nc.tensor.matmul(
                                psf[:M, :cw], ut_sb[t][:, q0:q0 + M], wo_t[t][:, c0:c0 + cw],
                                start=(t == 0), stop=(t == DT - 1))
                        nc.vector.tensor_add(
                            ob[:M, c0:c0 + cw], psf[:M, :cw], bo_sb[:M, c0:c0 + cw])
                    nc.sync.dma_start(out=out_d[q0:q0 + M, :], in_=ob[:M, :])

    nc.compile()
    return nc


_NC_CACHE = {}


def _get_nc():
    if "nc" not in _NC_CACHE:
        _NC_CACHE["nc"] = build_kernel()
    return _NC_CACHE["nc"]


def prep_in_maps(x, mask_weights, Wq, bq, Wk, bk, Wv, bv, Wo, bo, **_):
    x = np.asarray(x, dtype=np.float32)
    mask_weights = np.asarray(mask_weights, dtype=np.float32)
    Wq = np.asarray(Wq, np.float32); bq = np.asarray(bq, np.float32)
    Wk = np.asarray(Wk, np.float32); bk = np.asarray(bk, np.float32)
    Wv = np.asarray(Wv, np.float32); bv = np.asarray(bv, np.float32)
    Wo = np.asarray(Wo, np.float32); bo = np.asarray(bo, np.float32)

    scale = 1.0 / np.sqrt(np.float32(DH))
    xT = np.zeros((B, D, NP), np.float32)                        # zero-padded positions
    xT[:, :, :N] = x.transpose(0, 2, 1)
    wq_t = np.ascontiguousarray((Wq * scale).T)                  # (din, dout)
    wk_t = np.ascontiguousarray(Wk.T)
    wv_t = np.ascontiguousarray(Wv.T)
    wo_t = np.ascontiguousarray(Wo.T)
    bq_p = np.ascontiguousarray((bq * scale).reshape(DT, 128).T)  # (128, DT)
    bk_p = np.ascontiguousarray(bk.reshape(DT, 128).T)
    bo_eff = (bo + Wo @ bv).reshape(1, D).astype(np.float32)

    full_mask = np.concatenate(
        [np.ones((B, 1, 1), np.float32), mask_weights], axis=1)   # (B, N, 1)
    bias = np.empty((B, 2, NP), np.float32)
    bias[:, 0, :N] = (ALPHA * np.log(full_mask + np.float32(EPS))).reshape(B, N)
    bias[:, 0, N:] = -1e30                                        # padded keys: exp -> 0
    bias[:, 1, :] = 1.0                                           # ones row for the Q side

    in_maps = []
    for b in range(B):
        in_maps.append({
            "xT": xT[b], "wq": wq_t, "wk": wk_t, "wv": wv_t, "wo": wo_t,
            "bq": bq_p, "bk": bk_p, "bo": bo_eff, "bias": bias[b],
        })
    return in_maps


def kernel(**inputs):
    in_maps = prep_in_maps(**inputs)
    nc = _get_nc()
    res = run_bass_kernel_spmd(nc, in_maps, core_ids=list(range(B)))
    out = np.stack([r["out"] for r in res.results])
    attn = np.stack([r["attn"] for r in res.results])
    return out, attn
